# revision 41
# baseline (speedup 1.0000x reference)
"""Trainium2 Bass kernel for nn_Conv_SQT_22213570855264 (moe_routing).

Data-parallel over batch: 64 samples -> 8 NeuronCores x 8 samples.
Per-sample pipeline (all activations resident in SBUF, bf16 matmuls on PE,
fp32 PSUM accumulate):
  conv1 (routed, im2col K=54, host-side im2col) -> relu -> avgpool2
  (scale folded into conv2_w) -> conv2 3x3 -> relu -> mid1 5x5 -> relu
  -> mid2 5x5 -> relu -> convup2 (stride-2 transpose conv via 4 parity
  convs) -> tanh -> convup1 (routed 3x3, M=9, 4x column-tiled PE) -> +bias
  -> out [8,9,64,64] per core.
Expert routing (weight gather by i/o) is done host-side in numpy.
"""
import sys
import numpy as np

if '/opt/trn_rl_repo' not in sys.path:
    sys.path.insert(0, '/opt/trn_rl_repo')

B = 64
N_CORES = 8
BPC = B // N_CORES          # samples per core
TAPS3 = [(dy, dx) for dy in range(3) for dx in range(3)]
TAPS5 = [(dy, dx) for dy in range(5) for dx in range(5)]
# transpose-conv parity taps: parity -> [(delta, k)]
_TY = {0: [(0, 1)], 1: [(0, 0), (1, 2)]}


def _parity_taps():
    """[( (py,px), [(tapidx, dy, dx)] )] and the (ky,kx) per tapidx."""
    ptaps, kk = [], []
    ti = 0
    for py in (0, 1):
        for px in (0, 1):
            lst = []
            for (dly, ky) in _TY[py]:
                for (dlx, kx) in _TY[px]:
                    kk.append((ky, kx))
                    lst.append((ti, dly, dlx))
                    ti += 1
            ptaps.append(((py, px), lst))
    return ptaps, kk


_PTAPS, _PKK = _parity_taps()

_STATE = {}


def _build_nc():
    from concourse import bacc
    import concourse.mybir as mybir
    import concourse.tile as tile

    f32 = mybir.dt.float32
    bf16 = mybir.dt.bfloat16
    AF = mybir.ActivationFunctionType

    nc = bacc.Bacc("TRN2", target_bir_lowering=False, debug=False,
                   num_devices=N_CORES)

    i2c_e = nc.dram_tensor("i2c", [BPC, 54, 64, 64], bf16, kind="ExternalInput")
    w1_e = nc.dram_tensor("w1", [54, BPC, 128], bf16, kind="ExternalInput")
    b1_e = nc.dram_tensor("b1", [128, BPC], f32, kind="ExternalInput")
    w2_e = nc.dram_tensor("w2", [128, 9, 2, 128], bf16, kind="ExternalInput")
    # mid conv weights split by (mh, kh) so the first-needed halves load first
    wm1_e = [nc.dram_tensor(f"wm1h{h}", [128, 25, 128], bf16,
                            kind="ExternalInput") for h in range(4)]
    bm1_e = nc.dram_tensor("bm1", [128, 2], f32, kind="ExternalInput")
    wm2_e = [nc.dram_tensor(f"wm2h{h}", [128, 25, 128], bf16,
                            kind="ExternalInput") for h in range(4)]
    bm2_e = nc.dram_tensor("bm2", [128, 2], f32, kind="ExternalInput")
    wu2_e = nc.dram_tensor("wu2", [128, 9, 2, 128], bf16, kind="ExternalInput")
    bu2_e = nc.dram_tensor("bu2", [128, 1], f32, kind="ExternalInput")
    wu1_e = nc.dram_tensor("wu1", [128, BPC, 9, 9], bf16, kind="ExternalInput")
    bu1_e = nc.dram_tensor("bu1", [128, BPC], f32, kind="ExternalInput")
    out_e = nc.dram_tensor("out", [BPC, 9, 64, 64], f32, kind="ExternalOutput")

    with tile.TileContext(nc) as tc:
        with tc.tile_pool(name="w", bufs=1) as wp, \
             tc.tile_pool(name="a", bufs=1) as ap_, \
             tc.tile_pool(name="t", bufs=4) as tp, \
             tc.tile_pool(name="ps", bufs=8, space="PSUM") as pp:

            # ---- weights to SBUF (order = first-use order) ----
            w1t = wp.tile([54, BPC, 128], bf16)
            b1t = wp.tile([128, BPC], f32)
            w2t = wp.tile([128, 9, 2, 128], bf16)
            wm1t = [wp.tile([128, 25, 128], bf16, tag=f"wm1h{h}",
                            name=f"wm1h{h}") for h in range(4)]
            bm1t = wp.tile([128, 2], f32)
            wm2t = [wp.tile([128, 25, 128], bf16, tag=f"wm2h{h}",
                            name=f"wm2h{h}") for h in range(4)]
            bm2t = wp.tile([128, 2], f32)
            wu2t = wp.tile([128, 9, 2, 128], bf16)
            bu2t = wp.tile([128, 1], f32)
            wu1t = wp.tile([128, BPC, 9, 9], bf16)
            bu1t = wp.tile([128, BPC], f32)

            # ---- activation buffers (allocated once, zero borders persist) --
            i2c = ap_.tile([54, 64, 64], bf16)
            y1r = ap_.tile([128, 2, 8, 64], bf16)          # conv1 out ring
            y2 = ap_.tile([128, 34, 34], bf16)             # pooled, pad1
            y3 = [ap_.tile([128, 36, 36], bf16, tag=f"y3_{h}", name=f"y3_{h}") for h in range(2)]
            y4 = [ap_.tile([128, 36, 36], bf16, tag=f"y4_{h}", name=f"y4_{h}") for h in range(2)]
            y5 = [ap_.tile([128, 33, 33], bf16, tag=f"y5_{h}", name=f"y5_{h}") for h in range(2)]
            y6 = ap_.tile([128, 66, 66], bf16)             # tanh out, pad1
            # early preload: only what sample 0 needs soon, in need order.
            # One dma_start lands on one DMA ring (~20GB/s), so big loads
            # are split into several dma_starts to spread across rings; the
            # rest is deferred below so its HBM traffic doesn't delay
            # sample 0's inputs under fair DMA-ring sharing.
            nc.sync.dma_start(out=i2c[:, 0:8, :], in_=i2c_e[0, :, 0:8, :])
            nc.sync.dma_start(out=b1t[:], in_=b1_e[:])
            nc.sync.dma_start(out=w1t[:, 0:1, :], in_=w1_e[:, 0:1, :])
            nc.sync.dma_start(out=i2c[:, 8:32, :], in_=i2c_e[0, :, 8:32, :])
            nc.sync.dma_start(out=i2c[:, 32:64, :], in_=i2c_e[0, :, 32:64, :])
            nc.sync.dma_start(out=w1t[:, 1:BPC, :], in_=w1_e[:, 1:BPC, :])
            nc.sync.dma_start(out=w2t[:, 0:5], in_=w2_e[:, 0:5])
            nc.sync.dma_start(out=w2t[:, 5:9], in_=w2_e[:, 5:9])
            for h in range(4):
                nc.sync.dma_start(out=wm1t[h][:, 0:13], in_=wm1_e[h][:, 0:13])
                nc.sync.dma_start(out=wm1t[h][:, 13:25], in_=wm1_e[h][:, 13:25])
            nc.sync.dma_start(out=bm1t[:], in_=bm1_e[:])
            # zero pad borders via DVE broadcast copy - no HBM traffic
            ztiny = wp.tile([128, 1, 1], bf16)
            nc.gpsimd.memset(ztiny[:], 0.0)
            for z in [y2, y3[0], y3[1], y4[0], y4[1], y5[0], y5[1], y6]:
                nc.vector.tensor_copy(out=z[:],
                                      in_=ztiny[:].to_broadcast(z.shape))

            y6v = y6[:, 1:65, 1:65].rearrange(
                "p (r two) (c tw) -> p r two c tw", two=2, tw=2)

            def emit_i2c(b):
                nc.sync.dma_start(out=i2c[:], in_=i2c_e[b])

            def emit_conv1(b, chunks):
                for c in chunks:
                    r0 = 8 * c
                    ps = pp.tile([128, 8, 64], mybir.dt.float32, tag="ps")
                    nc.tensor.matmul(ps[:], w1t[:, b, :], i2c[:, r0:r0 + 8, :],
                                     start=True, stop=True)
                    slot = y1r[:, c % 2]
                    nc.scalar.activation(slot, ps[:], AF.Relu,
                                         bias=b1t[:, b:b + 1])
                    # avgpool 2x2 (scale folded into conv2 weights)
                    v = slot.rearrange("p (r two) (c tw) -> p r two c tw",
                                       two=2, tw=2)
                    ta = tp.tile([128, 4, 32], bf16, tag="poola")
                    tb = tp.tile([128, 4, 32], bf16, tag="poolb")
                    nc.vector.tensor_add(out=ta[:], in0=v[:, :, 0, :, 0],
                                         in1=v[:, :, 0, :, 1])
                    nc.vector.tensor_add(out=tb[:], in0=v[:, :, 1, :, 0],
                                         in1=v[:, :, 1, :, 1])
                    nc.vector.tensor_add(out=y2[:, 1 + 4 * c:5 + 4 * c, 1:33],
                                         in0=ta[:], in1=tb[:])

            def emit_conv2(b):
                # ck-outer: PSUM group ck0 finishes (and its act drains,
                # hidden under ck1's matmuls) before ck1 completes, so only
                # one activation is pending at the layer end.
                for mh in range(2):
                    pss = [pp.tile([128, 16, 32], mybir.dt.float32, tag="ps",
                                   name=f"ps{_}") for _ in range(2)]
                    for ck in range(2):
                        r0 = 16 * ck
                        for t, (dy, dx) in enumerate(TAPS3):
                            nc.tensor.matmul(
                                pss[ck][:], w2t[:, t, mh, :],
                                y2[:, dy + r0:dy + r0 + 16, dx:dx + 32],
                                start=(t == 0), stop=(t == 8))
                        nc.scalar.activation(
                            y3[mh][:, 2 + r0:18 + r0, 2:34], pss[ck][:],
                            AF.Relu)

            def emit_mid(yin, wt, bt, yout, oy, ox):
                # kh-blocked accumulation: all kh=0 taps first, so the first
                # ~50 matmuls only need yin[0] and the yin[1] drain latency
                # from the producing layer's mh=1 group is hidden.
                for mh in range(2):
                    pss = [pp.tile([128, 16, 32], mybir.dt.float32, tag="ps",
                                   name=f"ps{_}") for _ in range(2)]
                    for ck in range(2):
                        r0 = 16 * ck
                        for kh in range(2):
                            for ti, (dy, dx) in enumerate(TAPS5):
                                nc.tensor.matmul(
                                    pss[ck][:], wt[2 * mh + kh][:, ti, :],
                                    yin[kh][:, dy + r0:dy + r0 + 16,
                                            dx:dx + 32],
                                    start=(ti == 0 and kh == 0),
                                    stop=(ti == 24 and kh == 1))
                        nc.scalar.activation(
                            yout[mh][:, oy + r0:oy + r0 + 16, ox:ox + 32],
                            pss[ck][:], AF.Relu, bias=bt[:, mh:mh + 1])

            def emit_up2(b):
                for (py, px), lst in _PTAPS:
                    pss = [pp.tile([128, 16, 32], mybir.dt.float32, tag="ps",
                                   name=f"ps{_}") for _ in range(2)]
                    n = len(lst) * 2
                    for ck in range(2):
                        r0 = 16 * ck
                        cnt = 0
                        for kh in range(2):
                            for (ti, dly, dlx) in lst:
                                nc.tensor.matmul(
                                    pss[ck][:], wu2t[:, ti, kh, :],
                                    y5[kh][:, r0 + dly:r0 + dly + 16,
                                           dlx:dlx + 32],
                                    start=(cnt == 0), stop=(cnt == n - 1))
                                cnt += 1
                        nc.scalar.activation(
                            y6v[:, r0:r0 + 16, py, :, px], pss[ck][:],
                            AF.Tanh, bias=bu2t[:, 0:1])

            def emit_up1(b):
                # 4x column-tiled: chunks pass*4+j run concurrently on PE
                # column groups j (output partitions 32j..32j+8).
                for p in range(2):
                    ps = pp.tile([128, 8, 64], mybir.dt.float32, tag="ps")
                    st = tp.tile([128, 8, 64], f32, tag="ostage")
                    for t, (dy, dx) in enumerate(TAPS3):
                        for j in range(4):
                            r0 = 8 * (4 * p + j)
                            nc.tensor.matmul(
                                ps[32 * j:32 * j + 9], wu1t[:, b, t, :],
                                y6[:, dy + r0:dy + r0 + 8, dx:dx + 64],
                                start=(t == 0), stop=(t == 8),
                                tile_position=(0, 32 * j),
                                skip_group_check=True)
                    for j in range(4):
                        r0 = 8 * (4 * p + j)
                        nc.scalar.activation(st[32 * j:32 * j + 9],
                                             ps[32 * j:32 * j + 9],
                                             AF.Identity,
                                             bias=bu1t[32 * j:32 * j + 9,
                                                       b:b + 1])
                        nc.sync.dma_start(out=out_e[b, :, r0:r0 + 8, :],
                                          in_=st[32 * j:32 * j + 9])

            # software-pipelined schedule: conv1 of sample b+1 is emitted
            # into sample b's layer boundaries so the PE never waits on the
            # relu->pool->conv2 latency chain at sample starts.
            emit_conv1(0, range(8))
            for h in range(4):
                nc.sync.dma_start(out=wm2t[h][:], in_=wm2_e[h][:])
            for t_, e_ in [(bm2t, bm2_e),
                           (wu2t, wu2_e), (bu2t, bu2_e),
                           (wu1t, wu1_e), (bu1t, bu1_e)]:
                nc.sync.dma_start(out=t_[:], in_=e_[:])
            for b in range(BPC):
                emit_conv2(b)
                nxt = b + 1 if b + 1 < BPC else None
                if nxt is not None:
                    emit_i2c(nxt)
                emit_mid(y3, wm1t, bm1t, y4, 2, 2)
                if nxt is not None:
                    emit_conv1(nxt, range(0, 2))
                emit_mid(y4, wm2t, bm2t, y5, 0, 0)
                if nxt is not None:
                    emit_conv1(nxt, range(2, 4))
                emit_up2(b)
                if nxt is not None:
                    emit_conv1(nxt, range(4, 8))
                emit_up1(b)

    _dedup_ldweights(nc)
    nc.finalize()
    return nc


def _dedup_ldweights(nc):
    """Remove back-to-back InstLdweights that reload the identical weight AP
    (the ck-pair structure emits one per matmul).  The matmuls are already
    non-self-loading (ldweights=False) and the duplicate loads carry no sync
    info, so the later ones are pure PE-sequencer overhead."""
    import concourse.mybir as mybir
    removed = 0
    for fn in nc.m.functions:
        for bb in fn.blocks:
            lst = bb.instructions
            rm = []
            last_ldw_key = None
            for k, inst in enumerate(lst):
                if inst.engine != mybir.EngineType.PE:
                    continue
                if isinstance(inst, mybir.InstLdweights):
                    si = inst.sync_info
                    clean = si is None or (len(si.on_wait) == 0
                                           and len(si.on_update) == 0)
                    key = (str(inst.ins[0]), str(inst.tile_position),
                           str(inst.perf_mode), str(inst.is_transpose))
                    if clean and key == last_ldw_key:
                        rm.append(k)
                        continue
                    last_ldw_key = key
                elif not isinstance(inst, mybir.InstMatmult):
                    # any other PE instruction invalidates the loaded-weight
                    # assumption (stay conservative)
                    last_ldw_key = None
            for k in reversed(rm):
                del lst[k]
            removed += len(rm)
    return removed


def _get_nc():
    if 'nc' not in _STATE:
        _STATE['nc'] = _build_nc()
    return _STATE['nc']


def _prep_in_maps(inputs):
    import ml_dtypes
    bf16 = ml_dtypes.bfloat16
    f = lambda a: np.ascontiguousarray(np.asarray(a), dtype=np.float32)
    i_idx = np.asarray(inputs['i']).astype(np.int64)
    o_idx = np.asarray(inputs['o']).astype(np.int64)
    x0 = np.concatenate([f(inputs['in0']), f(inputs['in1'])], axis=1)  # [B,6,64,64]
    x = np.zeros((B, 6, 66, 66), np.float32)
    x[:, :, 1:65, 1:65] = x0
    # host-side im2col: [B, 54, 64, 64], tap-major (dy, dx, c)
    i2c = np.empty((B, 54, 64, 64), np.float32)
    for t, (dy, dx) in enumerate(TAPS3):
        i2c[:, 6 * t:6 * t + 6] = x[:, :, dy:dy + 64, dx:dx + 64]
    i2c = i2c.astype(bf16)

    W1 = f(inputs['conv1_w'])[i_idx]                       # [B,128,6,3,3]
    w1l = W1.transpose(0, 3, 4, 2, 1).reshape(B, 54, 128)  # [b,(dy,dx,c),m]
    b1 = f(inputs['conv1_b'])[i_idx]                       # [B,128]

    w2l = (f(inputs['conv2_w']) * 0.25).transpose(1, 2, 3, 0).reshape(
        128, 9, 2, 128).astype(bf16)

    def mid_layout(w):
        # -> [ (mh, kh) ][128k, 25tap, 128m]
        v = f(w).reshape(2, 128, 2, 128, 5, 5)  # [mh, m, kh, k, ky, kx]
        v = v.transpose(0, 2, 3, 4, 5, 1)       # [mh, kh, k, ky, kx, m]
        return [np.ascontiguousarray(v[mh, kh].reshape(128, 25, 128)).astype(bf16)
                for mh in range(2) for kh in range(2)]
    wm1 = mid_layout(inputs['conv_mid1_w'])
    wm2 = mid_layout(inputs['conv_mid2_w'])
    bm1 = f(inputs['conv_mid1_b']).reshape(2, 128).T       # [128,2]
    bm2 = f(inputs['conv_mid2_b']).reshape(2, 128).T

    wu2f = np.flip(f(inputs['convup2_w']), axis=(2, 3)).transpose(1, 0, 2, 3)
    wu2_arr = np.stack([wu2f[:, :, ky, kx] for (ky, kx) in _PKK])  # [9,128o,256i]
    wu2l = np.ascontiguousarray(
        wu2_arr.reshape(9, 128, 2, 128).transpose(3, 0, 2, 1)).astype(bf16)
    bu2 = f(inputs['convup2_b']).reshape(128, 1)

    Wg = np.flip(f(inputs['convup1_w'])[o_idx], axis=(3, 4)).transpose(
        0, 2, 1, 3, 4)                                     # [B,9,128,3,3]
    wu1l = np.ascontiguousarray(
        Wg.transpose(2, 0, 3, 4, 1).reshape(128, B, 9, 9)).astype(bf16)  # [k,b,tap,co]
    bu1 = f(inputs['convup1_b'])[o_idx]                    # [B,9]

    shared = {
        'w2': w2l,
        'bm1': np.ascontiguousarray(bm1),
        'bm2': np.ascontiguousarray(bm2),
        'wu2': wu2l,
        'bu2': np.ascontiguousarray(bu2),
    }
    for h in range(4):
        shared[f'wm1h{h}'] = wm1[h]
        shared[f'wm2h{h}'] = wm2[h]
    in_maps = []
    for c in range(N_CORES):
        s = slice(c * BPC, (c + 1) * BPC)
        m = dict(shared)
        m['i2c'] = np.ascontiguousarray(i2c[s])
        m['w1'] = np.ascontiguousarray(w1l[s].transpose(1, 0, 2)).astype(bf16)
        m['b1'] = np.ascontiguousarray(b1[s].T)
        m['wu1'] = np.ascontiguousarray(wu1l[:, s])
        bu1r = np.zeros((128, BPC), np.float32)
        for j in range(4):
            bu1r[32 * j:32 * j + 9, :] = bu1[s].T
        m['bu1'] = bu1r
        in_maps.append(m)
    return in_maps


def _run(inputs, trace=False):
    if trace:
        # shim the missing antenv.axon_hooks so NTFF profiling works
        import types
        try:
            import antenv.axon_hooks  # noqa
        except ImportError:
            from trn_agent_boot.trn_boot import _ntff_profile_via_ctypes
            hook = _ntff_profile_via_ctypes('/opt/axon/libaxon_pjrt.so')
            mod = types.ModuleType('antenv.axon_hooks')
            mod.get_axon_ntff_profile_hook = lambda: hook
            sys.modules['antenv.axon_hooks'] = mod
        from concourse import bass_utils
        bass_utils.upload_artifacts = lambda tmpdir: "local://" + tmpdir

    from concourse.bass_utils import run_bass_kernel_spmd
    nc = _get_nc()
    in_maps = _prep_in_maps(inputs)
    res = run_bass_kernel_spmd(nc, in_maps, list(range(N_CORES)), trace=trace)
    out = np.concatenate([res.results[c]['out'] for c in range(N_CORES)],
                         axis=0)                           # [64,9,64,64]
    o1, o2, o3 = out[:, 0:3], out[:, 3:6], out[:, 6:9]
    return (np.ascontiguousarray(o1), np.ascontiguousarray(o2),
            np.ascontiguousarray(o3)), res.exec_time_ns


def kernel(**inputs):
    outs, _ = _run(inputs, trace=False)
    return outs


# revision 49
# speedup vs baseline: 1.0027x; 1.0027x over previous
"""Trainium2 Bass kernel for nn_Conv_SQT_22213570855264 (moe_routing).

Data-parallel over batch: 64 samples -> 8 NeuronCores x 8 samples.
Per-sample pipeline (all activations resident in SBUF, bf16 matmuls on PE,
fp32 PSUM accumulate):
  conv1 (routed, im2col K=54, host-side im2col) -> relu -> avgpool2
  (scale folded into conv2_w) -> conv2 3x3 -> relu -> mid1 5x5 -> relu
  -> mid2 5x5 -> relu -> convup2 (stride-2 transpose conv via 4 parity
  convs) -> tanh -> convup1 (routed 3x3, M=9, 4x column-tiled PE) -> +bias
  -> out [8,9,64,64] per core.
Expert routing (weight gather by i/o) is done host-side in numpy.
"""
import sys
import numpy as np

if '/opt/trn_rl_repo' not in sys.path:
    sys.path.insert(0, '/opt/trn_rl_repo')

B = 64
N_CORES = 8
BPC = B // N_CORES          # samples per core
TAPS3 = [(dy, dx) for dy in range(3) for dx in range(3)]
TAPS5 = [(dy, dx) for dy in range(5) for dx in range(5)]
# transpose-conv parity taps: parity -> [(delta, k)]
_TY = {0: [(0, 1)], 1: [(0, 0), (1, 2)]}


def _parity_taps():
    """[( (py,px), [(tapidx, dy, dx)] )] and the (ky,kx) per tapidx."""
    ptaps, kk = [], []
    ti = 0
    for py in (0, 1):
        for px in (0, 1):
            lst = []
            for (dly, ky) in _TY[py]:
                for (dlx, kx) in _TY[px]:
                    kk.append((ky, kx))
                    lst.append((ti, dly, dlx))
                    ti += 1
            ptaps.append(((py, px), lst))
    return ptaps, kk


_PTAPS, _PKK = _parity_taps()

_STATE = {}


def _build_nc():
    from concourse import bacc
    import concourse.mybir as mybir
    import concourse.tile as tile

    f32 = mybir.dt.float32
    bf16 = mybir.dt.bfloat16
    AF = mybir.ActivationFunctionType

    nc = bacc.Bacc("TRN2", target_bir_lowering=False, debug=False,
                   num_devices=N_CORES)

    i2c_e = nc.dram_tensor("i2c", [BPC, 64, 64, 64], bf16, kind="ExternalInput")
    w1_e = nc.dram_tensor("w1", [128, BPC, 128], bf16, kind="ExternalInput")
    b1_e = nc.dram_tensor("b1", [128, BPC], f32, kind="ExternalInput")
    w2_e = nc.dram_tensor("w2", [128, 9, 2, 128], bf16, kind="ExternalInput")
    # mid conv weights split by (mh, kh) so the first-needed halves load first
    wm1_e = [nc.dram_tensor(f"wm1h{h}", [128, 25, 128], bf16,
                            kind="ExternalInput") for h in range(4)]
    bm1_e = nc.dram_tensor("bm1", [128, 2], f32, kind="ExternalInput")
    wm2_e = [nc.dram_tensor(f"wm2h{h}", [128, 25, 128], bf16,
                            kind="ExternalInput") for h in range(4)]
    bm2_e = nc.dram_tensor("bm2", [128, 2], f32, kind="ExternalInput")
    wu2_e = nc.dram_tensor("wu2", [128, 9, 2, 128], bf16, kind="ExternalInput")
    bu2_e = nc.dram_tensor("bu2", [128, 1], f32, kind="ExternalInput")
    wu1_e = nc.dram_tensor("wu1", [128, BPC, 9, 9], bf16, kind="ExternalInput")
    bu1_e = nc.dram_tensor("bu1", [128, BPC], f32, kind="ExternalInput")
    out_e = nc.dram_tensor("out", [BPC, 9, 64, 64], f32, kind="ExternalOutput")

    with tile.TileContext(nc) as tc:
        with tc.tile_pool(name="w", bufs=1) as wp, \
             tc.tile_pool(name="a", bufs=1) as ap_, \
             tc.tile_pool(name="t", bufs=4) as tp, \
             tc.tile_pool(name="ps", bufs=8, space="PSUM") as pp:

            # ---- weights to SBUF (order = first-use order) ----
            # K padded 54->128 with zeros: keeps conv1 matmuls in the same
            # 128-row PE configuration as every other layer, so the weight
            # pull-ahead pipeline never reconfigures (a row-group mask
            # change serializes LDWEIGHTS, ~+300ns per transition).
            w1t = wp.tile([128, BPC, 128], bf16)
            b1t = wp.tile([128, BPC], f32)
            w2t = wp.tile([128, 9, 2, 128], bf16)
            wm1t = [wp.tile([128, 25, 128], bf16, tag=f"wm1h{h}",
                            name=f"wm1h{h}") for h in range(4)]
            bm1t = wp.tile([128, 2], f32)
            wm2t = [wp.tile([128, 25, 128], bf16, tag=f"wm2h{h}",
                            name=f"wm2h{h}") for h in range(4)]
            bm2t = wp.tile([128, 2], f32)
            wu2t = wp.tile([128, 9, 2, 128], bf16)
            bu2t = wp.tile([128, 1], f32)
            wu1t = wp.tile([128, BPC, 9, 9], bf16)
            bu1t = wp.tile([128, BPC], f32)

            # ---- activation buffers (allocated once, zero borders persist) --
            i2c = ap_.tile([128, 64, 64], bf16)
            y1r = ap_.tile([128, 2, 8, 64], bf16)          # conv1 out ring
            y2 = ap_.tile([128, 34, 34], bf16)             # pooled, pad1
            y3 = [ap_.tile([128, 36, 36], bf16, tag=f"y3_{h}", name=f"y3_{h}") for h in range(2)]
            y4 = [ap_.tile([128, 36, 36], bf16, tag=f"y4_{h}", name=f"y4_{h}") for h in range(2)]
            y5 = [ap_.tile([128, 33, 33], bf16, tag=f"y5_{h}", name=f"y5_{h}") for h in range(2)]
            y6 = ap_.tile([128, 66, 66], bf16)             # tanh out, pad1
            # early preload: only what sample 0 needs soon, in need order.
            # One dma_start lands on one DMA ring (~20GB/s), so big loads
            # are split into several dma_starts to spread across rings; the
            # rest is deferred below so its HBM traffic doesn't delay
            # sample 0's inputs under fair DMA-ring sharing.
            nc.sync.dma_start(out=i2c[0:64, 0:8, :], in_=i2c_e[0, :, 0:8, :])
            nc.sync.dma_start(out=b1t[:], in_=b1_e[:])
            nc.sync.dma_start(out=w1t[:, 0:1, :], in_=w1_e[:, 0:1, :])
            nc.sync.dma_start(out=i2c[0:64, 8:32, :], in_=i2c_e[0, :, 8:32, :])
            nc.sync.dma_start(out=i2c[0:64, 32:64, :], in_=i2c_e[0, :, 32:64, :])
            nc.sync.dma_start(out=w1t[:, 1:BPC, :], in_=w1_e[:, 1:BPC, :])
            nc.sync.dma_start(out=w2t[:, 0:5], in_=w2_e[:, 0:5])
            nc.sync.dma_start(out=w2t[:, 5:9], in_=w2_e[:, 5:9])
            for h in range(4):
                nc.sync.dma_start(out=wm1t[h][:, 0:13], in_=wm1_e[h][:, 0:13])
                nc.sync.dma_start(out=wm1t[h][:, 13:25], in_=wm1_e[h][:, 13:25])
            nc.sync.dma_start(out=bm1t[:], in_=bm1_e[:])
            # zero pad borders via DVE broadcast copy - no HBM traffic
            ztiny = wp.tile([128, 1, 1], bf16)
            nc.gpsimd.memset(ztiny[:], 0.0)
            for z in [y2, y3[0], y3[1], y4[0], y4[1], y5[0], y5[1], y6]:
                nc.vector.tensor_copy(out=z[:],
                                      in_=ztiny[:].to_broadcast(z.shape))
            # zero the K-padding rows of the im2col buffer once (rows 54-63
            # are zero-padded host-side and re-DMAed per sample)
            nc.vector.tensor_copy(
                out=i2c[64:128],
                in_=ztiny[64:128].to_broadcast([64, 64, 64]))

            y6v = y6[:, 1:65, 1:65].rearrange(
                "p (r two) (c tw) -> p r two c tw", two=2, tw=2)

            def emit_i2c(b):
                nc.sync.dma_start(out=i2c[0:64], in_=i2c_e[b])

            def emit_conv1(b, chunks):
                for c in chunks:
                    r0 = 8 * c
                    ps = pp.tile([128, 8, 64], mybir.dt.float32, tag="ps")
                    nc.tensor.matmul(ps[:], w1t[:, b, :], i2c[:, r0:r0 + 8, :],
                                     start=True, stop=True)
                    slot = y1r[:, c % 2]
                    nc.scalar.activation(slot, ps[:], AF.Relu,
                                         bias=b1t[:, b:b + 1])
                    # avgpool 2x2 (scale folded into conv2 weights)
                    v = slot.rearrange("p (r two) (c tw) -> p r two c tw",
                                       two=2, tw=2)
                    ta = tp.tile([128, 4, 32], bf16, tag="poola")
                    tb = tp.tile([128, 4, 32], bf16, tag="poolb")
                    nc.vector.tensor_add(out=ta[:], in0=v[:, :, 0, :, 0],
                                         in1=v[:, :, 0, :, 1])
                    nc.vector.tensor_add(out=tb[:], in0=v[:, :, 1, :, 0],
                                         in1=v[:, :, 1, :, 1])
                    nc.vector.tensor_add(out=y2[:, 1 + 4 * c:5 + 4 * c, 1:33],
                                         in0=ta[:], in1=tb[:])

            def emit_conv2(b):
                # ck-outer: PSUM group ck0 finishes (and its act drains,
                # hidden under ck1's matmuls) before ck1 completes, so only
                # one activation is pending at the layer end.
                for mh in range(2):
                    pss = [pp.tile([128, 16, 32], mybir.dt.float32, tag="ps",
                                   name=f"ps{_}") for _ in range(2)]
                    for ck in range(2):
                        r0 = 16 * ck
                        for t, (dy, dx) in enumerate(TAPS3):
                            nc.tensor.matmul(
                                pss[ck][:], w2t[:, t, mh, :],
                                y2[:, dy + r0:dy + r0 + 16, dx:dx + 32],
                                start=(t == 0), stop=(t == 8))
                        nc.scalar.activation(
                            y3[mh][:, 2 + r0:18 + r0, 2:34], pss[ck][:],
                            AF.Relu)

            def emit_mid(yin, wt, bt, yout, oy, ox):
                # kh-blocked accumulation: all kh=0 taps first, so the first
                # ~50 matmuls only need yin[0] and the yin[1] drain latency
                # from the producing layer's mh=1 group is hidden.
                for mh in range(2):
                    pss = [pp.tile([128, 16, 32], mybir.dt.float32, tag="ps",
                                   name=f"ps{_}") for _ in range(2)]
                    for ck in range(2):
                        r0 = 16 * ck
                        for kh in range(2):
                            for ti, (dy, dx) in enumerate(TAPS5):
                                nc.tensor.matmul(
                                    pss[ck][:], wt[2 * mh + kh][:, ti, :],
                                    yin[kh][:, dy + r0:dy + r0 + 16,
                                            dx:dx + 32],
                                    start=(ti == 0 and kh == 0),
                                    stop=(ti == 24 and kh == 1))
                        nc.scalar.activation(
                            yout[mh][:, oy + r0:oy + r0 + 16, ox:ox + 32],
                            pss[ck][:], AF.Relu, bias=bt[:, mh:mh + 1])

            def emit_up2(b):
                for (py, px), lst in _PTAPS:
                    pss = [pp.tile([128, 16, 32], mybir.dt.float32, tag="ps",
                                   name=f"ps{_}") for _ in range(2)]
                    n = len(lst) * 2
                    for ck in range(2):
                        r0 = 16 * ck
                        cnt = 0
                        for kh in range(2):
                            for (ti, dly, dlx) in lst:
                                nc.tensor.matmul(
                                    pss[ck][:], wu2t[:, ti, kh, :],
                                    y5[kh][:, r0 + dly:r0 + dly + 16,
                                           dlx:dlx + 32],
                                    start=(cnt == 0), stop=(cnt == n - 1))
                                cnt += 1
                        nc.scalar.activation(
                            y6v[:, r0:r0 + 16, py, :, px], pss[ck][:],
                            AF.Tanh, bias=bu2t[:, 0:1])

            def emit_up1(b):
                # 4x column-tiled: chunks pass*4+j run concurrently on PE
                # column groups j (output partitions 32j..32j+8).
                for p in range(2):
                    ps = pp.tile([128, 8, 64], mybir.dt.float32, tag="ps")
                    st = tp.tile([128, 8, 64], f32, tag="ostage")
                    for t, (dy, dx) in enumerate(TAPS3):
                        for j in range(4):
                            r0 = 8 * (4 * p + j)
                            nc.tensor.matmul(
                                ps[32 * j:32 * j + 9], wu1t[:, b, t, :],
                                y6[:, dy + r0:dy + r0 + 8, dx:dx + 64],
                                start=(t == 0), stop=(t == 8),
                                tile_position=(0, 32 * j),
                                skip_group_check=True)
                    for j in range(4):
                        r0 = 8 * (4 * p + j)
                        nc.scalar.activation(st[32 * j:32 * j + 9],
                                             ps[32 * j:32 * j + 9],
                                             AF.Identity,
                                             bias=bu1t[32 * j:32 * j + 9,
                                                       b:b + 1])
                        nc.sync.dma_start(out=out_e[b, :, r0:r0 + 8, :],
                                          in_=st[32 * j:32 * j + 9])

            # software-pipelined schedule: conv1 of sample b+1 is emitted
            # into sample b's layer boundaries so the PE never waits on the
            # relu->pool->conv2 latency chain at sample starts.
            emit_conv1(0, range(8))
            for h in range(4):
                nc.sync.dma_start(out=wm2t[h][:], in_=wm2_e[h][:])
            for t_, e_ in [(bm2t, bm2_e),
                           (wu2t, wu2_e), (bu2t, bu2_e),
                           (wu1t, wu1_e), (bu1t, bu1_e)]:
                nc.sync.dma_start(out=t_[:], in_=e_[:])
            for b in range(BPC):
                emit_conv2(b)
                nxt = b + 1 if b + 1 < BPC else None
                if nxt is not None:
                    emit_i2c(nxt)
                emit_mid(y3, wm1t, bm1t, y4, 2, 2)
                if nxt is not None:
                    emit_conv1(nxt, range(0, 2))
                emit_mid(y4, wm2t, bm2t, y5, 0, 0)
                if nxt is not None:
                    emit_conv1(nxt, range(2, 4))
                emit_up2(b)
                if nxt is not None:
                    emit_conv1(nxt, range(4, 8))
                emit_up1(b)

    _dedup_ldweights(nc)
    nc.finalize()
    return nc


def _dedup_ldweights(nc):
    """Remove back-to-back InstLdweights that reload the identical weight AP
    (the ck-pair structure emits one per matmul).  The matmuls are already
    non-self-loading (ldweights=False) and the duplicate loads carry no sync
    info, so the later ones are pure PE-sequencer overhead."""
    import concourse.mybir as mybir
    removed = 0
    for fn in nc.m.functions:
        for bb in fn.blocks:
            lst = bb.instructions
            rm = []
            last_ldw_key = None
            for k, inst in enumerate(lst):
                if inst.engine != mybir.EngineType.PE:
                    continue
                if isinstance(inst, mybir.InstLdweights):
                    si = inst.sync_info
                    clean = si is None or (len(si.on_wait) == 0
                                           and len(si.on_update) == 0)
                    key = (str(inst.ins[0]), str(inst.tile_position),
                           str(inst.perf_mode), str(inst.is_transpose))
                    if clean and key == last_ldw_key:
                        rm.append(k)
                        continue
                    last_ldw_key = key
                elif not isinstance(inst, mybir.InstMatmult):
                    # any other PE instruction invalidates the loaded-weight
                    # assumption (stay conservative)
                    last_ldw_key = None
            for k in reversed(rm):
                del lst[k]
            removed += len(rm)
    return removed


def _get_nc():
    if 'nc' not in _STATE:
        _STATE['nc'] = _build_nc()
    return _STATE['nc']


def _prep_in_maps(inputs):
    import ml_dtypes
    bf16 = ml_dtypes.bfloat16
    f = lambda a: np.ascontiguousarray(np.asarray(a), dtype=np.float32)
    i_idx = np.asarray(inputs['i']).astype(np.int64)
    o_idx = np.asarray(inputs['o']).astype(np.int64)
    x0 = np.concatenate([f(inputs['in0']), f(inputs['in1'])], axis=1)  # [B,6,64,64]
    x = np.zeros((B, 6, 66, 66), np.float32)
    x[:, :, 1:65, 1:65] = x0
    # host-side im2col: [B, 64, 64, 64], tap-major (dy, dx, c), K padded
    # 54->64 with zeros (rows 64-127 of the SBUF tile are zeroed on-chip)
    i2c = np.zeros((B, 64, 64, 64), np.float32)
    for t, (dy, dx) in enumerate(TAPS3):
        i2c[:, 6 * t:6 * t + 6] = x[:, :, dy:dy + 64, dx:dx + 64]
    i2c = i2c.astype(bf16)

    W1 = f(inputs['conv1_w'])[i_idx]                       # [B,128,6,3,3]
    w1l = np.zeros((B, 128, 128), np.float32)              # K padded 54->128
    w1l[:, 0:54] = W1.transpose(0, 3, 4, 2, 1).reshape(B, 54, 128)
    b1 = f(inputs['conv1_b'])[i_idx]                       # [B,128]

    w2l = (f(inputs['conv2_w']) * 0.25).transpose(1, 2, 3, 0).reshape(
        128, 9, 2, 128).astype(bf16)

    def mid_layout(w):
        # -> [ (mh, kh) ][128k, 25tap, 128m]
        v = f(w).reshape(2, 128, 2, 128, 5, 5)  # [mh, m, kh, k, ky, kx]
        v = v.transpose(0, 2, 3, 4, 5, 1)       # [mh, kh, k, ky, kx, m]
        return [np.ascontiguousarray(v[mh, kh].reshape(128, 25, 128)).astype(bf16)
                for mh in range(2) for kh in range(2)]
    wm1 = mid_layout(inputs['conv_mid1_w'])
    wm2 = mid_layout(inputs['conv_mid2_w'])
    bm1 = f(inputs['conv_mid1_b']).reshape(2, 128).T       # [128,2]
    bm2 = f(inputs['conv_mid2_b']).reshape(2, 128).T

    wu2f = np.flip(f(inputs['convup2_w']), axis=(2, 3)).transpose(1, 0, 2, 3)
    wu2_arr = np.stack([wu2f[:, :, ky, kx] for (ky, kx) in _PKK])  # [9,128o,256i]
    wu2l = np.ascontiguousarray(
        wu2_arr.reshape(9, 128, 2, 128).transpose(3, 0, 2, 1)).astype(bf16)
    bu2 = f(inputs['convup2_b']).reshape(128, 1)

    Wg = np.flip(f(inputs['convup1_w'])[o_idx], axis=(3, 4)).transpose(
        0, 2, 1, 3, 4)                                     # [B,9,128,3,3]
    wu1l = np.ascontiguousarray(
        Wg.transpose(2, 0, 3, 4, 1).reshape(128, B, 9, 9)).astype(bf16)  # [k,b,tap,co]
    bu1 = f(inputs['convup1_b'])[o_idx]                    # [B,9]

    shared = {
        'w2': w2l,
        'bm1': np.ascontiguousarray(bm1),
        'bm2': np.ascontiguousarray(bm2),
        'wu2': wu2l,
        'bu2': np.ascontiguousarray(bu2),
    }
    for h in range(4):
        shared[f'wm1h{h}'] = wm1[h]
        shared[f'wm2h{h}'] = wm2[h]
    in_maps = []
    for c in range(N_CORES):
        s = slice(c * BPC, (c + 1) * BPC)
        m = dict(shared)
        m['i2c'] = np.ascontiguousarray(i2c[s])
        m['w1'] = np.ascontiguousarray(w1l[s].transpose(1, 0, 2)).astype(bf16)
        m['b1'] = np.ascontiguousarray(b1[s].T)
        m['wu1'] = np.ascontiguousarray(wu1l[:, s])
        bu1r = np.zeros((128, BPC), np.float32)
        for j in range(4):
            bu1r[32 * j:32 * j + 9, :] = bu1[s].T
        m['bu1'] = bu1r
        in_maps.append(m)
    return in_maps


def _run(inputs, trace=False):
    if trace:
        # shim the missing antenv.axon_hooks so NTFF profiling works
        import types
        try:
            import antenv.axon_hooks  # noqa
        except ImportError:
            from trn_agent_boot.trn_boot import _ntff_profile_via_ctypes
            hook = _ntff_profile_via_ctypes('/opt/axon/libaxon_pjrt.so')
            mod = types.ModuleType('antenv.axon_hooks')
            mod.get_axon_ntff_profile_hook = lambda: hook
            sys.modules['antenv.axon_hooks'] = mod
        from concourse import bass_utils
        bass_utils.upload_artifacts = lambda tmpdir: "local://" + tmpdir

    from concourse.bass_utils import run_bass_kernel_spmd
    nc = _get_nc()
    in_maps = _prep_in_maps(inputs)
    res = run_bass_kernel_spmd(nc, in_maps, list(range(N_CORES)), trace=trace)
    out = np.concatenate([res.results[c]['out'] for c in range(N_CORES)],
                         axis=0)                           # [64,9,64,64]
    o1, o2, o3 = out[:, 0:3], out[:, 3:6], out[:, 6:9]
    return (np.ascontiguousarray(o1), np.ascontiguousarray(o2),
            np.ascontiguousarray(o3)), res.exec_time_ns


def kernel(**inputs):
    outs, _ = _run(inputs, trace=False)
    return outs
